# revision 23
# baseline (speedup 1.0000x reference)
"""Trainium2 Bass kernel for the temporal-shift multi-head attention module.

Sharding: data-parallel over the video axis — 8 videos of 8 frames each,
one video (8 frames x 197 tokens) per NeuronCore. The temporal head shift
only moves data between frames of the same video, so it is a pure slicing
operation on-device. Weights are replicated. No collectives.

Per-core pipeline (all on-chip, bf16 matmul operands, fp32 accumulation):
  1. DMA x naturally, cast bf16 (DVE), PE-transpose 6 column tiles into ONE
     shared PSUM bank per m-tile, evict with a single 3D-AP DVE copy into
     one channel-major xTall tile.
  2. Adapter: hT = aw1^T @ xT (+b1, ACT evict); xT += aw2^T @ hT (+b2, DVE
     scalar_tensor_tensor, in place).
  3. qT/kT = W^T @ x1T (channel-major; bias-evict alternating ACT/DVE);
     v = x1T^T @ Wv per frame token-major, evicted as a PURE COPY
     (alternating engines): the v bias is folded into the projection bias
     b_eff = qkv_b[v] @ proj_w + proj_b, computed on-device once.
  4. Attention per frame, per head pair (pairs share the shifted frame):
     - scoresT: both key j-tiles of one head go in ONE PSUM bank
       [128, 394]; kT is zero-padded by 64 tokens so the second j-tile
       spans a full 128 stationary columns. Head pairs are row-tiled
       (kT/qT partitions 0:64 vs 64:128) so their matmuls run
       concurrently on the PE array.
     - ONE exp per head (ACT, [128, 394], scale folded in).
     - softmax denominators: 24 accumulating one-hot matmuls into one
       [12,197] bank, one batched DVE reciprocal per frame, DMA bounce
       through DRAM, TWO broadcast DMAs per frame rebuild a pair-interleaved
       [128, 6*197] reciprocal tile (partition p<64 = even head of pair).
     - av: per pair ONE PSUM bank [128,197]; even head -> partitions 0:64,
       odd head -> 64:128 (col-tiled, concurrent), then a single DVE
       multiply per pair writes aoT directly.
  5. proj interleaved per frame: out = aoT^T @ Wp + b_eff, DMA out.

Hardware pitfalls encoded here: matmul weight APs must have one free dim;
per PSUM bank use exactly one start=True (first matmul touching it) and one
stop=True — later matmuls to disjoint byte ranges overwrite via the
pending-zero mechanism; DMA cannot read PSUM; SBUF APs cannot
partition-broadcast (DRAM sources can); custom-DVE ops crash this runtime.
DMA queues: weight staging goes on the ACT queue, everything latency-
critical (softmax bounce) plus loads/stores on the sync queue.
"""

import numpy as np

F = 8
N = 197
C = 768
HADP = 192
NH = 12
HD = 64
M = F * N  # 1576
SCALE = HD ** -0.5
NCORES = 8
MCHUNKS = [(0, 512), (512, 512), (1024, 512), (1536, 40)]
MTILES = [(i * 128, 128) for i in range(12)] + [(1536, 40)]
JTILES = [(0, 128), (128, 69)]
KPAD = 64
N2 = 2 * N

_CACHE = {}


def _build():
    import concourse.mybir as mybir
    from concourse import bacc
    import concourse.tile as tile
    from concourse.masks import make_identity

    BF = mybir.dt.bfloat16
    FP = mybir.dt.float32
    AT = mybir.ActivationFunctionType
    OP = mybir.AluOpType

    nc = bacc.Bacc("TRN2", target_bir_lowering=False, debug=False)

    x_e = nc.dram_tensor("x", [F, N, C], FP, kind="ExternalInput")
    aw1_e = nc.dram_tensor("a_w1", [C, HADP], FP, kind="ExternalInput")
    ab1_e = nc.dram_tensor("a_b1", [HADP], FP, kind="ExternalInput")
    aw2_e = nc.dram_tensor("a_w2", [HADP, C], FP, kind="ExternalInput")
    ab2_e = nc.dram_tensor("a_b2", [C], FP, kind="ExternalInput")
    qkvw_e = nc.dram_tensor("qkv_w", [C, 3 * C], FP, kind="ExternalInput")
    qkvb_e = nc.dram_tensor("qkv_b", [3 * C], FP, kind="ExternalInput")
    projw_e = nc.dram_tensor("proj_w", [C, C], FP, kind="ExternalInput")
    projb_e = nc.dram_tensor("proj_b", [C], FP, kind="ExternalInput")
    out_e = nc.dram_tensor("out", [F, N, C], FP, kind="ExternalOutput")

    xf = x_e.rearrange("f n c -> (f n) c")
    outf = out_e.rearrange("f n c -> (f n) c")

    with tile.TileContext(nc) as tc:
        with tc.tile_pool(name="persist", bufs=1) as pp, \
             tc.tile_pool(name="scratch", bufs=2) as sp, \
             tc.tile_pool(name="drp", bufs=2, space="DRAM") as drp:
            # ---- constants
            ident = pp.tile([128, 128], FP, name="ident", tag="ident")
            make_identity(nc, ident)
            identB = pp.tile([128, 128], BF, name="identB", tag="identB")
            make_identity(nc, identB)

            # ---- persistent activations
            xTall = pp.tile([128, 6 * M], BF, name="xTall", tag="xTall")

            def xT(ct, lo, sz):
                return xTall[:, ct * M + lo: ct * M + lo + sz]

            qT = [pp.tile([128, M], BF, name=f"qT{i}", tag=f"qT{i}")
                  for i in range(6)]
            kT = [pp.tile([128, M + KPAD], BF, name=f"kT{i}", tag=f"kT{i}")
                  for i in range(6)]
            aoT = [pp.tile([128, M], BF, name=f"aoT{i}", tag=f"aoT{i}")
                   for i in range(6)]
            vt = [[pp.tile([128, C], BF, name=f"v{f}_{j}", tag=f"v{f}_{j}")
                   for j in range(2)] for f in range(F)]
            for i in range(6):
                nc.vector.memset(kT[i][:, M:M + KPAD], 0.0)
            oneblock = pp.tile([128, NH * NH], BF, name="oneblock", tag="oneblock")
            nc.vector.memset(oneblock[:, :], 0.0)
            for h in range(NH):
                nc.vector.memset(oneblock[:, h * NH + h:h * NH + h + 1], 1.0)

            # ---- phase 1: load x, cast bf16 (DVE), PE-transpose 6 ct-tiles
            # into one shared PSUM bank per m-tile, single 3D-AP DVE evict.
            # x staging has its OWN slots (separate from weight staging) so
            # the weight-load pipeline runs fully concurrent on other queues.
            warm_w = pp.tile([128, 64], BF, name="warm_w", tag="warm_w")
            nc.vector.memset(warm_w[:, :], 1.0)
            with tc.tile_pool(name="pst", bufs=3, space="PSUM") as pst:
                # ~5us burst of tiny matmuls while the first x DMA is in
                # flight: trips the HAM activity monitor so the PE clock is
                # already 2.4 GHz when real matmuls arrive (transposes alone
                # do not register as PE activity for the clock gate).
                for wi in range(90):
                    wps = pst.tile([64, 64], FP, bufs=2, name=f"warm{wi}",
                                   tag="warm")
                    nc.tensor.matmul(wps[:, :], warm_w[:, 0:64], warm_w[:, 0:64],
                                     start=True, stop=True)
                for mt, (mb, msz) in enumerate(MTILES):
                    xn = sp.tile([128, C], FP, bufs=4, name=f"xn{mt}", tag="xstg")
                    eng = nc.sync if mt % 2 == 0 else nc.scalar
                    eng.dma_start(xn[0:msz, :], xf[mb:mb + msz, :])
                    # transpose straight from fp32 (cayman transpose_mode is
                    # full-rate for fp32); fp32 PSUM targets (4+2 ct-tiles per
                    # bank), the bf16 cast happens in the DVE eviction
                    xv = xTall[:, :].rearrange("p (c m) -> p c m", m=M)
                    for g, (c0, cn) in enumerate([(0, 4), (4, 2)]):
                        pt = pst.tile([128, cn * 128], FP,
                                      name=f"pt{mt}_{g}", tag=f"pt{g}")
                        for ci in range(cn):
                            ct = c0 + ci
                            nc.tensor.matmul(pt[:, ci * 128:ci * 128 + msz],
                                             xn[0:msz, ct * 128:(ct + 1) * 128],
                                             ident[0:msz, 0:msz],
                                             is_transpose=True,
                                             start=(ci == 0), stop=(ci == cn - 1))
                        src = pt[:, :].rearrange("p (c k) -> p c k", k=128)[:, :, 0:msz]
                        nc.vector.tensor_copy(xv[:, c0:c0 + cn, mb:mb + msz], src)

            # ---- weights: chunked DMA fp32 staging striped across BOTH DMA
            # queues (two independent slot pipelines) -> bf16 via DVE casts
            _wn = [0]

            def load_cast(name, p, fdim, src_ap):
                dstw = pp.tile([p, fdim], BF, name=name, tag=name)
                for cb in range(0, fdim, C):
                    csz = min(C, fdim - cb)
                    lane = _wn[0] % 2
                    stg = sp.tile([128, C], FP, bufs=2,
                                  name=f"stg_{name}_{cb}", tag=f"wstg{lane}")
                    (nc.scalar if lane else nc.sync).dma_start(
                        stg[0:p, 0:csz], src_ap[:, cb:cb + csz])
                    nc.vector.tensor_copy(dstw[:, cb:cb + csz], stg[0:p, 0:csz])
                    _wn[0] += 1
                return dstw

            aw1 = [load_cast(f"aw1_{k}", 128, HADP,
                             aw1_e[k * 128:(k + 1) * 128, :]) for k in range(6)]
            aw2 = [load_cast("aw2_0", 128, C, aw2_e[0:128, :]),
                   load_cast("aw2_1", 64, C, aw2_e[128:HADP, :])]
            qkvw = [load_cast(f"qkvw{k}", 128, 3 * C,
                              qkvw_e[k * 128:(k + 1) * 128, :]) for k in range(6)]
            # biases: consolidated single DMAs ((o p) -> p o column layouts)
            b1c0 = pp.tile([128, 1], FP, name="b1c0", tag="b1c0")
            nc.sync.dma_start(b1c0[:, :], ab1_e[0:128][:, None])
            b1c1 = pp.tile([64, 1], FP, name="b1c1", tag="b1c1")
            nc.sync.dma_start(b1c1[:, :], ab1_e[128:HADP][:, None])
            b1c = [b1c0, b1c1]
            b2_all = pp.tile([128, 6], FP, name="b2_all", tag="b2_all")
            nc.sync.dma_start(b2_all[:, :],
                              ab2_e[:].rearrange("(o p) -> p o", p=128))
            qkb_all = pp.tile([128, 12], FP, name="qkb_all", tag="qkb_all")
            nc.sync.dma_start(qkb_all[:, :],
                              qkvb_e[0:1536].rearrange("(o p) -> p o", p=128))
            bv_all = pp.tile([128, 6], FP, name="bv_all", tag="bv_all")
            nc.sync.dma_start(bv_all[:, :],
                              qkvb_e[1536:2304].rearrange("(o p) -> p o", p=128))
            b2c = [b2_all[:, i:i + 1] for i in range(6)]
            qkbc = [qkb_all[:, i:i + 1] for i in range(12)]
            bvb_all = pp.tile([128, 6], BF, name="bvb_all", tag="bvb_all")
            nc.gpsimd.tensor_copy(bvb_all[:, :], bv_all[:, :])
            bvb = [bvb_all[:, i:i + 1] for i in range(6)]
            beff_sb = pp.tile([1, C], FP, name="beff_sb", tag="beff_sb")
            nc.sync.dma_start(beff_sb[0:1, :], projb_e[None, :])
            pbb = pp.tile([128, C], FP, name="pbb", tag="pbb")
            beff_d = drp.tile([1, C], FP, name="beff_d", tag="beff_d")

            # ---- phases 2-4
            with tc.tile_pool(name="psA", bufs=4, space="PSUM") as psA:
                # adapter hT
                hT = [sp.tile([128, M], BF, bufs=1, name="hT0", tag="hT0"),
                      sp.tile([64, M], BF, bufs=1, name="hT1", tag="hT1")]
                for mb, msz in MCHUNKS:
                    for ht, (hb, hsz) in enumerate([(0, 128), (128, 64)]):
                        ps = psA.tile([128, 512], FP, name=f"psh{ht}_{mb}", tag="psA")
                        for kt in range(6):
                            nc.tensor.matmul(ps[0:hsz, 0:msz],
                                             aw1[kt][:, hb:hb + hsz],
                                             xT(kt, mb, msz),
                                             start=(kt == 0), stop=(kt == 5))
                        nc.scalar.activation(hT[ht][:, mb:mb + msz], ps[0:hsz, 0:msz],
                                             AT.Identity, bias=b1c[ht][:, :])
                # x1 = x + adapter out (+b2), in place into xTall; the
                # residual is accumulated on the PE via an identity matmul so
                # the eviction is a cheap 2-op-free bias add, not a 3-operand
                # DVE scalar_tensor_tensor on the critical path to qkv
                for mb, msz in MCHUNKS:
                    for ct in range(6):
                        ps = psA.tile([128, 512], FP, name=f"psx{ct}_{mb}", tag="psA")
                        nc.tensor.matmul(ps[:, 0:msz], identB[:, :],
                                         xT(ct, mb, msz), start=True, stop=False)
                        for kt, ksz in enumerate([128, 64]):
                            nc.tensor.matmul(ps[:, 0:msz],
                                             aw2[kt][0:ksz, ct * 128:(ct + 1) * 128],
                                             hT[kt][0:ksz, mb:mb + msz],
                                             start=False, stop=(kt == 1))
                        if (ct + (mb // 512)) % 2 == 0:
                            nc.vector.tensor_scalar_add(xT(ct, mb, msz),
                                                        ps[:, 0:msz], b2c[ct])
                        else:
                            nc.scalar.activation(xT(ct, mb, msz), ps[:, 0:msz],
                                                 AT.Identity, bias=b2c[ct])

                # qT / kT (channel-major), bias-evict alternating ACT/DVE
                _evn = [0]
                for mb, msz in MCHUNKS:
                    for ot in range(12):
                        dstT = qT[ot] if ot < 6 else kT[ot - 6]
                        ps = psA.tile([128, 512], FP, name=f"psqk{ot}_{mb}", tag="psA")
                        for kt in range(6):
                            nc.tensor.matmul(ps[:, 0:msz],
                                             qkvw[kt][:, ot * 128:(ot + 1) * 128],
                                             xT(kt, mb, msz),
                                             start=(kt == 0), stop=(kt == 5))
                        if _evn[0] % 2 == 0:
                            nc.scalar.activation(dstT[:, mb:mb + msz], ps[:, 0:msz],
                                                 AT.Identity, bias=qkbc[ot])
                        else:
                            nc.vector.tensor_scalar_add(dstT[:, mb:mb + msz],
                                                        ps[:, 0:msz],
                                                        qkbc[ot])
                        _evn[0] += 1

                # v (token-major, per frame) — pure copy evict, no bias
                for f in range(F):
                    for jt, (jb, jsz) in enumerate(JTILES):
                        for half in range(2):
                            ps = psA.tile([128, 512], FP,
                                          name=f"psv{f}_{jt}_{half}", tag="psA")
                            for kt in range(6):
                                nc.tensor.matmul(
                                    ps[0:jsz, 0:384],
                                    xT(kt, f * N + jb, jsz),
                                    qkvw[kt][:, 1536 + half * 384:1536 + (half + 1) * 384],
                                    start=(kt == 0), stop=(kt == 5))
                            dstv = vt[f][jt][0:jsz, half * 384:(half + 1) * 384]
                            if (2 * f + jt + half) % 2 == 0:
                                nc.vector.tensor_copy(dstv, ps[0:jsz, 0:384])
                            else:
                                nc.scalar.copy(dstv, ps[0:jsz, 0:384])

                # proj weights loaded last (nothing needs them until the
                # attention phase) so their casts don't sit ahead of the
                # adapter/qkv work in the DVE queue
                projw = [load_cast(f"projw{k}", 128, C,
                                   projw_e[k * 128:(k + 1) * 128, :])
                         for k in range(6)]
                # b_eff = qkv_b[v-part] @ proj_w + proj_b (v bias folded
                # through the projection; exact identity since softmax rows
                # sum to one after normalization)
                for half in range(2):
                    bp = psA.tile([1, 384], FP, bufs=2, name=f"bp{half}",
                                  tag="psbeff")
                    for kt in range(6):
                        nc.tensor.matmul(bp[:, :], bvb[kt],
                                         projw[kt][:, half * 384:(half + 1) * 384],
                                         start=(kt == 0), stop=(kt == 5))
                    nc.vector.tensor_tensor(
                        out=beff_sb[0:1, half * 384:(half + 1) * 384],
                        in0=bp[:, :],
                        in1=beff_sb[0:1, half * 384:(half + 1) * 384],
                        op=OP.add)
                nc.scalar.dma_start(beff_d[0:1, :], beff_sb[0:1, :])
                nc.scalar.dma_start(pbb[:, :],
                                    beff_d[0, :][None, :].broadcast_to((128, C)))

            # ---- attention
            def fk_of(f, h):
                if h < 2:
                    return max(f - 1, 0)
                if h < 4:
                    return min(f + 1, F - 1)
                return f

            # process frame 7 before 6 so its two tail proj tiles (11, 12)
            # fire one frame early; tail after the last frame is then 2 tiles
            FSEQ = [0, 1, 2, 3, 4, 5, 7, 6]
            PROJMAP = {}
            _fired = set()
            for _pos, _f in enumerate(FSEQ):
                _done = set(FSEQ[:_pos + 1])
                PROJMAP[_f] = []
                for _mt, (_mb, _msz) in enumerate(MTILES):
                    if _mt in _fired:
                        continue
                    if set(range(_mb // N, (_mb + _msz - 1) // N + 1)) <= _done:
                        PROJMAP[_f].append(_mt)
                        _fired.add(_mt)

            with tc.tile_pool(name="psT", bufs=1, space="PSUM") as psT:
                for f in FSEQ:
                    es = []
                    # two concurrent den chains (col-tiled): jt0 sums in rows
                    # 0:12, jt1 sums in rows 32:44; summed on DVE before recip
                    dent = psT.tile([44, N], FP, bufs=1, name=f"den{f}", tag="den")

                    def den_mms(h):
                        for jt, (jb, jsz) in enumerate(JTILES):
                            nc.tensor.matmul(
                                dent[32 * jt:32 * jt + NH, :],
                                oneblock[0:jsz, h * NH:(h + 1) * NH],
                                es[h][0:jsz, jt * N:jt * N + N],
                                start=(h == 0), stop=(h == 11))

                    # av per pair: one bank, col-tiled heads. Raw sums are
                    # evicted to SBUF immediately (frees the bank — the PE
                    # never waits on the reciprocal round-trip), the
                    # normalizing multiply happens later from SBUF.
                    avs = []

                    def av_pair(b):
                        fk2 = fk_of(f, 2 * b)
                        avp = psT.tile([128, N], FP, bufs=2, name=f"av{f}_{b}",
                                       tag="av")
                        for jt, (jb, jsz) in enumerate(JTILES):
                            for hi in range(2):
                                h = 2 * b + hi
                                nc.tensor.matmul(
                                    avp[hi * 64:(hi + 1) * 64, :],
                                    vt[fk2][jt][0:jsz, h * HD:(h + 1) * HD],
                                    es[h][0:jsz, jt * N:jt * N + N],
                                    start=(jt == 0), stop=(jt == 1))
                        asb = sp.tile([128, N], BF, bufs=7, name=f"avs{f}_{b}",
                                      tag="avs")
                        nc.vector.tensor_copy(asb[:, :], avp[:, :])
                        avs.append(asb)

                    for hp in range(6):
                        fk = fk_of(f, 2 * hp)
                        base = fk * N
                        psH = [psT.tile([128, N2], FP, bufs=3,
                                        name=f"st{f}_{hp}_{hi}", tag="st")
                               for hi in range(2)]
                        # A-jt0, B-jt0 (concurrent row groups), A-jt1, B-jt1
                        for jt in range(2):
                            for hi in range(2):
                                pb = hi * 64
                                nc.tensor.matmul(
                                    psH[hi][:, jt * N:(jt + 1) * N],
                                    kT[hp][pb:pb + 64, base + jt * 128:base + jt * 128 + 128],
                                    qT[hp][pb:pb + 64, f * N:(f + 1) * N],
                                    start=(jt == 0), stop=(jt == 1))
                        for hi in range(2):
                            h = 2 * hp + hi
                            e = sp.tile([128, N2], BF, bufs=16,
                                        name=f"e{f}_{h}", tag="e")
                            nc.scalar.activation(e[:, :], psH[hi][:, :],
                                                 AT.Exp, scale=SCALE)
                            es.append(e)
                        # den + av for the PREVIOUS pair: keeps the exp
                        # dependency a full pair behind the PE queue head and
                        # gives the PE av work during the exp-bound stretch
                        if hp >= 1:
                            den_mms(2 * hp - 2)
                            den_mms(2 * hp - 1)
                            av_pair(hp - 1)
                    den_mms(10)
                    den_mms(11)
                    av_pair(5)
                    denb = sp.tile([NH, N], FP, bufs=2, name=f"dnb{f}", tag="dnb")
                    nc.scalar.copy(denb[:, :], dent[32:32 + NH, :])
                    den_sum = sp.tile([NH, N], FP, bufs=2, name=f"dsum{f}",
                                      tag="dsum")
                    nc.vector.tensor_tensor(out=den_sum[:, :], in0=dent[0:NH, :],
                                            in1=denb[:, :], op=OP.add)
                    rec12 = sp.tile([NH, N], BF, bufs=2, name=f"rcp{f}", tag="rec12")
                    with nc.allow_low_precision(reason="bf16 softmax reciprocal"):
                        nc.vector.reciprocal(rec12[:, :], den_sum[:, :])
                    dr12 = drp.tile([NH, N], BF, name=f"dr12_{f}", tag="dr12")
                    nc.sync.dma_start(dr12[:, :], rec12[:, :])
                    # pair-interleaved broadcast: partitions 0:64 = even head
                    # of each pair, 64:128 = odd head
                    rec_all = sp.tile([128, 6 * N], BF, bufs=2,
                                      name=f"recall{f}", tag="recall")
                    for hi in range(2):
                        dstr = rec_all[hi * 64:(hi + 1) * 64, :].rearrange(
                            "p (b q) -> p b q", q=N)
                        srcr = dr12[hi::2, :][None, :, :].broadcast_to((64, 6, N))
                        nc.sync.dma_start(dstr, srcr)
                    for b in range(6):
                        nc.vector.tensor_tensor(
                            out=aoT[b][:, f * N:(f + 1) * N],
                            in0=avs[b][:, :],
                            in1=rec_all[:, b * N:(b + 1) * N],
                            op=OP.mult)
                    # proj for m-tiles fully covered by frames <= f
                    for mt in PROJMAP[f]:
                        mb, msz = MTILES[mt]
                        osb = sp.tile([128, C], FP, bufs=2, name=f"osb{mt}",
                                      tag="osb")
                        for half in range(2):
                            ps = psT.tile([128, 384], FP, bufs=2,
                                          name=f"psp{mt}_{half}", tag="psP")
                            for kt in range(6):
                                nc.tensor.matmul(
                                    ps[0:msz, :],
                                    aoT[kt][:, mb:mb + msz],
                                    projw[kt][:, half * 384:(half + 1) * 384],
                                    start=(kt == 0), stop=(kt == 5))
                            nc.vector.tensor_tensor(
                                out=osb[0:msz, half * 384:(half + 1) * 384],
                                in0=ps[0:msz, :],
                                in1=pbb[0:msz, half * 384:(half + 1) * 384],
                                op=OP.add)
                        nc.sync.dma_start(outf[mb:mb + msz, :], osb[0:msz, :])

    nc.compile()
    return nc


def _get_nc():
    if "nc" not in _CACHE:
        _CACHE["nc"] = _build()
    return _CACHE["nc"]


def _in_maps(inputs):
    x = np.ascontiguousarray(np.asarray(inputs["x"], np.float32))
    w = {k: np.ascontiguousarray(np.asarray(inputs[k], np.float32))
         for k in ("a_w1", "a_b1", "a_w2", "a_b2", "qkv_w", "qkv_b",
                   "proj_w", "proj_b")}
    maps = []
    for i in range(NCORES):
        m = {"x": x[i * F:(i + 1) * F]}
        m.update(w)
        maps.append(m)
    return maps


def kernel(**inputs):
    from concourse.bass_utils import run_bass_kernel_spmd
    nc = _get_nc()
    res = run_bass_kernel_spmd(nc, _in_maps(inputs), core_ids=list(range(NCORES)))
    return np.concatenate([res.results[i]["out"] for i in range(NCORES)], axis=0)


def run_traced(inputs, **kwargs):
    """Test harness helper: run with NTFF profiling, return (output, results)."""
    from concourse.bass_utils import run_bass_kernel_spmd
    nc = _get_nc()
    res = run_bass_kernel_spmd(nc, _in_maps(inputs),
                               core_ids=list(range(NCORES)), trace=True, **kwargs)
    out = np.concatenate([res.results[i]["out"] for i in range(NCORES)], axis=0)
    return out, res


# revision 24
# speedup vs baseline: 1.1744x; 1.1744x over previous
"""Trainium2 Bass kernel for the temporal-shift multi-head attention module.

Sharding: data-parallel over the video axis — 8 videos of 8 frames each,
one video (8 frames x 197 tokens) per NeuronCore. The temporal head shift
only moves data between frames of the same video, so it is a pure slicing
operation on-device. Weights are replicated. No collectives.

Per-core pipeline (all on-chip, bf16 matmul operands, fp32 accumulation):
  1. DMA x naturally, cast bf16 (DVE), PE-transpose 6 column tiles into ONE
     shared PSUM bank per m-tile, evict with a single 3D-AP DVE copy into
     one channel-major xTall tile.
  2. Adapter: hT = aw1^T @ xT (+b1, ACT evict); xT += aw2^T @ hT (+b2, DVE
     scalar_tensor_tensor, in place).
  3. qT/kT = W^T @ x1T (channel-major; bias-evict alternating ACT/DVE);
     v = x1T^T @ Wv per frame token-major, evicted as a PURE COPY
     (alternating engines): the v bias is folded into the projection bias
     b_eff = qkv_b[v] @ proj_w + proj_b, computed on-device once.
  4. Attention per frame, per head pair (pairs share the shifted frame):
     - scoresT: both key j-tiles of one head go in ONE PSUM bank
       [128, 394]; kT is zero-padded by 64 tokens so the second j-tile
       spans a full 128 stationary columns. Head pairs are row-tiled
       (kT/qT partitions 0:64 vs 64:128) so their matmuls run
       concurrently on the PE array.
     - ONE exp per head (ACT, [128, 394], scale folded in).
     - softmax denominators: 24 accumulating one-hot matmuls into one
       [12,197] bank, one batched DVE reciprocal per frame, DMA bounce
       through DRAM, TWO broadcast DMAs per frame rebuild a pair-interleaved
       [128, 6*197] reciprocal tile (partition p<64 = even head of pair).
     - av: per pair ONE PSUM bank [128,197]; even head -> partitions 0:64,
       odd head -> 64:128 (col-tiled, concurrent), then a single DVE
       multiply per pair writes aoT directly.
  5. proj interleaved per frame: out = aoT^T @ Wp + b_eff, DMA out.

Hardware pitfalls encoded here: matmul weight APs must have one free dim;
per PSUM bank use exactly one start=True (first matmul touching it) and one
stop=True — later matmuls to disjoint byte ranges overwrite via the
pending-zero mechanism; DMA cannot read PSUM; SBUF APs cannot
partition-broadcast (DRAM sources can); custom-DVE ops crash this runtime.
DMA queues: weight staging goes on the ACT queue, everything latency-
critical (softmax bounce) plus loads/stores on the sync queue.
"""

import numpy as np

F = 8
N = 197
C = 768
HADP = 192
NH = 12
HD = 64
M = F * N  # 1576
SCALE = HD ** -0.5
NCORES = 8
MCHUNKS = [(0, 512), (512, 512), (1024, 512), (1536, 40)]
MTILES = [(i * 128, 128) for i in range(12)] + [(1536, 40)]
JTILES = [(0, 128), (128, 69)]
KPAD = 64
N2 = 2 * N

_CACHE = {}


def _build():
    import concourse.mybir as mybir
    from concourse import bacc
    import concourse.tile as tile
    from concourse.masks import make_identity

    BF = mybir.dt.bfloat16
    FP = mybir.dt.float32
    AT = mybir.ActivationFunctionType
    OP = mybir.AluOpType

    nc = bacc.Bacc("TRN2", target_bir_lowering=False, debug=False)

    x_e = nc.dram_tensor("x", [F, N, C], FP, kind="ExternalInput")
    aw1_e = nc.dram_tensor("a_w1", [C, HADP], FP, kind="ExternalInput")
    ab1_e = nc.dram_tensor("a_b1", [HADP], FP, kind="ExternalInput")
    aw2_e = nc.dram_tensor("a_w2", [HADP, C], FP, kind="ExternalInput")
    ab2_e = nc.dram_tensor("a_b2", [C], FP, kind="ExternalInput")
    qkvw_e = nc.dram_tensor("qkv_w", [C, 3 * C], FP, kind="ExternalInput")
    qkvb_e = nc.dram_tensor("qkv_b", [3 * C], FP, kind="ExternalInput")
    projw_e = nc.dram_tensor("proj_w", [C, C], FP, kind="ExternalInput")
    projb_e = nc.dram_tensor("proj_b", [C], FP, kind="ExternalInput")
    out_e = nc.dram_tensor("out", [F, N, C], FP, kind="ExternalOutput")

    xf = x_e.rearrange("f n c -> (f n) c")
    outf = out_e.rearrange("f n c -> (f n) c")

    with tile.TileContext(nc) as tc:
        with tc.tile_pool(name="persist", bufs=1) as pp, \
             tc.tile_pool(name="scratch", bufs=2) as sp, \
             tc.tile_pool(name="drp", bufs=2, space="DRAM") as drp:
            # ---- constants
            ident = pp.tile([128, 128], FP, name="ident", tag="ident")
            make_identity(nc, ident)
            identB = pp.tile([128, 128], BF, name="identB", tag="identB")
            make_identity(nc, identB)

            # ---- persistent activations
            xTall = pp.tile([128, 6 * M], BF, name="xTall", tag="xTall")

            def xT(ct, lo, sz):
                return xTall[:, ct * M + lo: ct * M + lo + sz]

            qT = [pp.tile([128, M], BF, name=f"qT{i}", tag=f"qT{i}")
                  for i in range(6)]
            kT = [pp.tile([128, M + KPAD], BF, name=f"kT{i}", tag=f"kT{i}")
                  for i in range(6)]
            aoT = [pp.tile([128, M], BF, name=f"aoT{i}", tag=f"aoT{i}")
                   for i in range(6)]
            vt = [[pp.tile([128, C], BF, name=f"v{f}_{j}", tag=f"v{f}_{j}")
                   for j in range(2)] for f in range(F)]
            for i in range(6):
                nc.vector.memset(kT[i][:, M:M + KPAD], 0.0)
            oneblock = pp.tile([128, NH * NH], BF, name="oneblock", tag="oneblock")
            nc.vector.memset(oneblock[:, :], 0.0)
            for h in range(NH):
                nc.vector.memset(oneblock[:, h * NH + h:h * NH + h + 1], 1.0)

            # ---- phase 1: load x, cast bf16 (DVE), PE-transpose 6 ct-tiles
            # into one shared PSUM bank per m-tile, single 3D-AP DVE evict.
            # x staging has its OWN slots (separate from weight staging) so
            # the weight-load pipeline runs fully concurrent on other queues.
            warm_w = pp.tile([128, 64], BF, name="warm_w", tag="warm_w")
            nc.vector.memset(warm_w[:, :], 1.0)
            with tc.tile_pool(name="pst", bufs=3, space="PSUM") as pst:
                # ~5us burst of tiny matmuls while the first x DMA is in
                # flight: trips the HAM activity monitor so the PE clock is
                # already 2.4 GHz when real matmuls arrive (transposes alone
                # do not register as PE activity for the clock gate).
                for wi in range(90):
                    wps = pst.tile([64, 64], FP, bufs=2, name=f"warm{wi}",
                                   tag="warm")
                    nc.tensor.matmul(wps[:, :], warm_w[:, 0:64], warm_w[:, 0:64],
                                     start=True, stop=True)
                for mt, (mb, msz) in enumerate(MTILES):
                    xn = sp.tile([128, C], FP, bufs=4, name=f"xn{mt}", tag="xstg")
                    eng = nc.sync if mt % 2 == 0 else nc.scalar
                    eng.dma_start(xn[0:msz, :], xf[mb:mb + msz, :])
                    # transpose straight from fp32 (cayman transpose_mode is
                    # full-rate for fp32); fp32 PSUM targets (4+2 ct-tiles per
                    # bank), the bf16 cast happens in the DVE eviction
                    xv = xTall[:, :].rearrange("p (c m) -> p c m", m=M)
                    for g, (c0, cn) in enumerate([(0, 4), (4, 2)]):
                        pt = pst.tile([128, cn * 128], FP,
                                      name=f"pt{mt}_{g}", tag=f"pt{g}")
                        for ci in range(cn):
                            ct = c0 + ci
                            nc.tensor.matmul(pt[:, ci * 128:ci * 128 + msz],
                                             xn[0:msz, ct * 128:(ct + 1) * 128],
                                             ident[0:msz, 0:msz],
                                             is_transpose=True,
                                             start=(ci == 0), stop=(ci == cn - 1))
                        src = pt[:, :].rearrange("p (c k) -> p c k", k=128)[:, :, 0:msz]
                        nc.vector.tensor_copy(xv[:, c0:c0 + cn, mb:mb + msz], src)

            # ---- weights: chunked DMA fp32 staging striped across BOTH DMA
            # queues (two independent slot pipelines) -> bf16 via DVE casts
            _wn = [0]

            def load_cast(name, p, fdim, src_ap):
                dstw = pp.tile([p, fdim], BF, name=name, tag=name)
                for cb in range(0, fdim, C):
                    csz = min(C, fdim - cb)
                    lane = _wn[0] % 2
                    stg = sp.tile([128, C], FP, bufs=2,
                                  name=f"stg_{name}_{cb}", tag=f"wstg{lane}")
                    (nc.scalar if lane else nc.sync).dma_start(
                        stg[0:p, 0:csz], src_ap[:, cb:cb + csz])
                    nc.vector.tensor_copy(dstw[:, cb:cb + csz], stg[0:p, 0:csz])
                    _wn[0] += 1
                return dstw

            aw1 = [load_cast(f"aw1_{k}", 128, HADP,
                             aw1_e[k * 128:(k + 1) * 128, :]) for k in range(6)]
            aw2 = [load_cast("aw2_0", 128, C, aw2_e[0:128, :]),
                   load_cast("aw2_1", 64, C, aw2_e[128:HADP, :])]
            qkvw = [load_cast(f"qkvw{k}", 128, 3 * C,
                              qkvw_e[k * 128:(k + 1) * 128, :]) for k in range(6)]
            # biases: consolidated single DMAs ((o p) -> p o column layouts)
            b1c0 = pp.tile([128, 1], FP, name="b1c0", tag="b1c0")
            nc.sync.dma_start(b1c0[:, :], ab1_e[0:128][:, None])
            b1c1 = pp.tile([64, 1], FP, name="b1c1", tag="b1c1")
            nc.sync.dma_start(b1c1[:, :], ab1_e[128:HADP][:, None])
            b1c = [b1c0, b1c1]
            b2_all = pp.tile([128, 6], FP, name="b2_all", tag="b2_all")
            nc.sync.dma_start(b2_all[:, :],
                              ab2_e[:].rearrange("(o p) -> p o", p=128))
            qkb_all = pp.tile([128, 12], FP, name="qkb_all", tag="qkb_all")
            nc.sync.dma_start(qkb_all[:, :],
                              qkvb_e[0:1536].rearrange("(o p) -> p o", p=128))
            bv_all = pp.tile([128, 6], FP, name="bv_all", tag="bv_all")
            nc.sync.dma_start(bv_all[:, :],
                              qkvb_e[1536:2304].rearrange("(o p) -> p o", p=128))
            b2c = [b2_all[:, i:i + 1] for i in range(6)]
            qkbc = [qkb_all[:, i:i + 1] for i in range(12)]
            bvb_all = pp.tile([128, 6], BF, name="bvb_all", tag="bvb_all")
            nc.gpsimd.tensor_copy(bvb_all[:, :], bv_all[:, :])
            bvb = [bvb_all[:, i:i + 1] for i in range(6)]
            beff_sb = pp.tile([1, C], FP, name="beff_sb", tag="beff_sb")
            nc.sync.dma_start(beff_sb[0:1, :], projb_e[None, :])
            pbb = pp.tile([128, C], FP, name="pbb", tag="pbb")
            beff_d = drp.tile([1, C], FP, name="beff_d", tag="beff_d")

            # ---- phases 2-4
            with tc.tile_pool(name="psA", bufs=4, space="PSUM") as psA:
                # adapter hT
                hT = [sp.tile([128, M], BF, bufs=1, name="hT0", tag="hT0"),
                      sp.tile([64, M], BF, bufs=1, name="hT1", tag="hT1")]
                for ht, (hb, hsz) in enumerate([(0, 128), (128, 64)]):
                    for mb, msz in MCHUNKS:
                        ps = psA.tile([128, 512], FP, name=f"psh{ht}_{mb}", tag="psA")
                        for kt in range(6):
                            nc.tensor.matmul(ps[0:hsz, 0:msz],
                                             aw1[kt][:, hb:hb + hsz],
                                             xT(kt, mb, msz),
                                             start=(kt == 0), stop=(kt == 5))
                        nc.scalar.activation(hT[ht][:, mb:mb + msz], ps[0:hsz, 0:msz],
                                             AT.Identity, bias=b1c[ht][:, :])
                # x1 = x + adapter out (+b2), in place into xTall; the
                # residual is accumulated on the PE via an identity matmul so
                # the eviction is a cheap 2-op-free bias add, not a 3-operand
                # DVE scalar_tensor_tensor on the critical path to qkv
                for ct in range(6):
                    for mb, msz in MCHUNKS:
                        ps = psA.tile([128, 512], FP, name=f"psx{ct}_{mb}", tag="psA")
                        nc.tensor.matmul(ps[:, 0:msz], identB[:, :],
                                         xT(ct, mb, msz), start=True, stop=False)
                        for kt, ksz in enumerate([128, 64]):
                            nc.tensor.matmul(ps[:, 0:msz],
                                             aw2[kt][0:ksz, ct * 128:(ct + 1) * 128],
                                             hT[kt][0:ksz, mb:mb + msz],
                                             start=False, stop=(kt == 1))
                        if (ct + (mb // 512)) % 2 == 0:
                            nc.vector.tensor_scalar_add(xT(ct, mb, msz),
                                                        ps[:, 0:msz], b2c[ct])
                        else:
                            nc.scalar.activation(xT(ct, mb, msz), ps[:, 0:msz],
                                                 AT.Identity, bias=b2c[ct])

                # qT / kT (channel-major), bias-evict alternating ACT/DVE
                _evn = [0]
                for ot in range(12):
                    dstT = qT[ot] if ot < 6 else kT[ot - 6]
                    for mb, msz in MCHUNKS:
                        ps = psA.tile([128, 512], FP, name=f"psqk{ot}_{mb}", tag="psA")
                        for kt in range(6):
                            nc.tensor.matmul(ps[:, 0:msz],
                                             qkvw[kt][:, ot * 128:(ot + 1) * 128],
                                             xT(kt, mb, msz),
                                             start=(kt == 0), stop=(kt == 5))
                        if _evn[0] % 2 == 0:
                            nc.scalar.activation(dstT[:, mb:mb + msz], ps[:, 0:msz],
                                                 AT.Identity, bias=qkbc[ot])
                        else:
                            nc.vector.tensor_scalar_add(dstT[:, mb:mb + msz],
                                                        ps[:, 0:msz],
                                                        qkbc[ot])
                        _evn[0] += 1

                # v (token-major, per frame) — pure copy evict, no bias
                for f in range(F):
                    for jt, (jb, jsz) in enumerate(JTILES):
                        for half in range(2):
                            ps = psA.tile([128, 512], FP,
                                          name=f"psv{f}_{jt}_{half}", tag="psA")
                            for kt in range(6):
                                nc.tensor.matmul(
                                    ps[0:jsz, 0:384],
                                    xT(kt, f * N + jb, jsz),
                                    qkvw[kt][:, 1536 + half * 384:1536 + (half + 1) * 384],
                                    start=(kt == 0), stop=(kt == 5))
                            dstv = vt[f][jt][0:jsz, half * 384:(half + 1) * 384]
                            if (2 * f + jt + half) % 2 == 0:
                                nc.vector.tensor_copy(dstv, ps[0:jsz, 0:384])
                            else:
                                nc.scalar.copy(dstv, ps[0:jsz, 0:384])

                # proj weights loaded last (nothing needs them until the
                # attention phase) so their casts don't sit ahead of the
                # adapter/qkv work in the DVE queue
                projw = [load_cast(f"projw{k}", 128, C,
                                   projw_e[k * 128:(k + 1) * 128, :])
                         for k in range(6)]
                # b_eff = qkv_b[v-part] @ proj_w + proj_b (v bias folded
                # through the projection; exact identity since softmax rows
                # sum to one after normalization)
                for half in range(2):
                    bp = psA.tile([1, 384], FP, bufs=2, name=f"bp{half}",
                                  tag="psbeff")
                    for kt in range(6):
                        nc.tensor.matmul(bp[:, :], bvb[kt],
                                         projw[kt][:, half * 384:(half + 1) * 384],
                                         start=(kt == 0), stop=(kt == 5))
                    nc.vector.tensor_tensor(
                        out=beff_sb[0:1, half * 384:(half + 1) * 384],
                        in0=bp[:, :],
                        in1=beff_sb[0:1, half * 384:(half + 1) * 384],
                        op=OP.add)
                nc.scalar.dma_start(beff_d[0:1, :], beff_sb[0:1, :])
                nc.scalar.dma_start(pbb[:, :],
                                    beff_d[0, :][None, :].broadcast_to((128, C)))

            # ---- attention
            def fk_of(f, h):
                if h < 2:
                    return max(f - 1, 0)
                if h < 4:
                    return min(f + 1, F - 1)
                return f

            # process frame 7 before 6 so its two tail proj tiles (11, 12)
            # fire one frame early; tail after the last frame is then 2 tiles
            FSEQ = [0, 1, 2, 3, 4, 5, 7, 6]
            PROJMAP = {}
            _fired = set()
            for _pos, _f in enumerate(FSEQ):
                _done = set(FSEQ[:_pos + 1])
                PROJMAP[_f] = []
                for _mt, (_mb, _msz) in enumerate(MTILES):
                    if _mt in _fired:
                        continue
                    if set(range(_mb // N, (_mb + _msz - 1) // N + 1)) <= _done:
                        PROJMAP[_f].append(_mt)
                        _fired.add(_mt)

            with tc.tile_pool(name="psT", bufs=1, space="PSUM") as psT:
                for f in FSEQ:
                    es = []
                    # two concurrent den chains (col-tiled): jt0 sums in rows
                    # 0:12, jt1 sums in rows 32:44; summed on DVE before recip
                    dent = psT.tile([44, N], FP, bufs=1, name=f"den{f}", tag="den")

                    def den_mms(h):
                        for jt, (jb, jsz) in enumerate(JTILES):
                            nc.tensor.matmul(
                                dent[32 * jt:32 * jt + NH, :],
                                oneblock[0:jsz, h * NH:(h + 1) * NH],
                                es[h][0:jsz, jt * N:jt * N + N],
                                start=(h == 0), stop=(h == 11))

                    # av per pair: one bank, col-tiled heads. Raw sums are
                    # evicted to SBUF immediately (frees the bank — the PE
                    # never waits on the reciprocal round-trip), the
                    # normalizing multiply happens later from SBUF.
                    avs = []

                    def av_pair(b):
                        fk2 = fk_of(f, 2 * b)
                        avp = psT.tile([128, N], FP, bufs=2, name=f"av{f}_{b}",
                                       tag="av")
                        for jt, (jb, jsz) in enumerate(JTILES):
                            for hi in range(2):
                                h = 2 * b + hi
                                nc.tensor.matmul(
                                    avp[hi * 64:(hi + 1) * 64, :],
                                    vt[fk2][jt][0:jsz, h * HD:(h + 1) * HD],
                                    es[h][0:jsz, jt * N:jt * N + N],
                                    start=(jt == 0), stop=(jt == 1))
                        asb = sp.tile([128, N], BF, bufs=7, name=f"avs{f}_{b}",
                                      tag="avs")
                        nc.vector.tensor_copy(asb[:, :], avp[:, :])
                        avs.append(asb)

                    for hp in range(6):
                        fk = fk_of(f, 2 * hp)
                        base = fk * N
                        psH = [psT.tile([128, N2], FP, bufs=3,
                                        name=f"st{f}_{hp}_{hi}", tag="st")
                               for hi in range(2)]
                        # A-jt0, B-jt0 (concurrent row groups), A-jt1, B-jt1
                        for jt in range(2):
                            for hi in range(2):
                                pb = hi * 64
                                nc.tensor.matmul(
                                    psH[hi][:, jt * N:(jt + 1) * N],
                                    kT[hp][pb:pb + 64, base + jt * 128:base + jt * 128 + 128],
                                    qT[hp][pb:pb + 64, f * N:(f + 1) * N],
                                    start=(jt == 0), stop=(jt == 1))
                        for hi in range(2):
                            h = 2 * hp + hi
                            e = sp.tile([128, N2], BF, bufs=16,
                                        name=f"e{f}_{h}", tag="e")
                            nc.scalar.activation(e[:, :], psH[hi][:, :],
                                                 AT.Exp, scale=SCALE)
                            es.append(e)
                        # den + av for the PREVIOUS pair: keeps the exp
                        # dependency a full pair behind the PE queue head and
                        # gives the PE av work during the exp-bound stretch
                        if hp >= 1:
                            den_mms(2 * hp - 2)
                            den_mms(2 * hp - 1)
                            av_pair(hp - 1)
                    den_mms(10)
                    den_mms(11)
                    av_pair(5)
                    denb = sp.tile([NH, N], FP, bufs=2, name=f"dnb{f}", tag="dnb")
                    nc.scalar.copy(denb[:, :], dent[32:32 + NH, :])
                    den_sum = sp.tile([NH, N], FP, bufs=2, name=f"dsum{f}",
                                      tag="dsum")
                    nc.vector.tensor_tensor(out=den_sum[:, :], in0=dent[0:NH, :],
                                            in1=denb[:, :], op=OP.add)
                    rec12 = sp.tile([NH, N], BF, bufs=2, name=f"rcp{f}", tag="rec12")
                    with nc.allow_low_precision(reason="bf16 softmax reciprocal"):
                        nc.vector.reciprocal(rec12[:, :], den_sum[:, :])
                    dr12 = drp.tile([NH, N], BF, name=f"dr12_{f}", tag="dr12")
                    nc.sync.dma_start(dr12[:, :], rec12[:, :])
                    # pair-interleaved broadcast: partitions 0:64 = even head
                    # of each pair, 64:128 = odd head
                    rec_all = sp.tile([128, 6 * N], BF, bufs=2,
                                      name=f"recall{f}", tag="recall")
                    for hi in range(2):
                        dstr = rec_all[hi * 64:(hi + 1) * 64, :].rearrange(
                            "p (b q) -> p b q", q=N)
                        srcr = dr12[hi::2, :][None, :, :].broadcast_to((64, 6, N))
                        nc.sync.dma_start(dstr, srcr)
                    for b in range(6):
                        nc.vector.tensor_tensor(
                            out=aoT[b][:, f * N:(f + 1) * N],
                            in0=avs[b][:, :],
                            in1=rec_all[:, b * N:(b + 1) * N],
                            op=OP.mult)
                    # proj for m-tiles fully covered by frames <= f
                    for mt in PROJMAP[f]:
                        mb, msz = MTILES[mt]
                        osb = sp.tile([128, C], FP, bufs=2, name=f"osb{mt}",
                                      tag="osb")
                        for half in range(2):
                            ps = psT.tile([128, 384], FP, bufs=2,
                                          name=f"psp{mt}_{half}", tag="psP")
                            for kt in range(6):
                                nc.tensor.matmul(
                                    ps[0:msz, :],
                                    aoT[kt][:, mb:mb + msz],
                                    projw[kt][:, half * 384:(half + 1) * 384],
                                    start=(kt == 0), stop=(kt == 5))
                            nc.vector.tensor_tensor(
                                out=osb[0:msz, half * 384:(half + 1) * 384],
                                in0=ps[0:msz, :],
                                in1=pbb[0:msz, half * 384:(half + 1) * 384],
                                op=OP.add)
                        nc.sync.dma_start(outf[mb:mb + msz, :], osb[0:msz, :])

    nc.compile()
    return nc


def _get_nc():
    if "nc" not in _CACHE:
        _CACHE["nc"] = _build()
    return _CACHE["nc"]


def _in_maps(inputs):
    x = np.ascontiguousarray(np.asarray(inputs["x"], np.float32))
    w = {k: np.ascontiguousarray(np.asarray(inputs[k], np.float32))
         for k in ("a_w1", "a_b1", "a_w2", "a_b2", "qkv_w", "qkv_b",
                   "proj_w", "proj_b")}
    maps = []
    for i in range(NCORES):
        m = {"x": x[i * F:(i + 1) * F]}
        m.update(w)
        maps.append(m)
    return maps


def kernel(**inputs):
    from concourse.bass_utils import run_bass_kernel_spmd
    nc = _get_nc()
    res = run_bass_kernel_spmd(nc, _in_maps(inputs), core_ids=list(range(NCORES)))
    return np.concatenate([res.results[i]["out"] for i in range(NCORES)], axis=0)


def run_traced(inputs, **kwargs):
    """Test harness helper: run with NTFF profiling, return (output, results)."""
    from concourse.bass_utils import run_bass_kernel_spmd
    nc = _get_nc()
    res = run_bass_kernel_spmd(nc, _in_maps(inputs),
                               core_ids=list(range(NCORES)), trace=True, **kwargs)
    out = np.concatenate([res.results[i]["out"] for i in range(NCORES)], axis=0)
    return out, res
